# revision 5
# baseline (speedup 1.0000x reference)
"""MentionScore fused Bass kernel for 8 Trainium2 NeuronCores.

Strategy (self-contained, hardcoded for the nn_MentionScore problem):
  - Spans are bucketed by start//6250 -> one bucket per core; each core only
    needs its 6250-token slice (+9 halo), so states/embeds are sharded with
    no collectives.
  - Token phase (feature-major): per-token attention-logit MLP -> e = exp(a);
    the span MLP's first layer is decomposed through the gathers:
        g @ sw1 = A[start] + B[end] + (pooled @ sw1_p) + width-term
    with A = states@sw1[0:400], B = states@sw1[400:800], and
    pooled @ sw1_p = (P[end] - P[start-1]) / (Pe[end] - Pe[start-1]) where
    [P | Pe] = inclusive prefix sums of [e*EC | e], EC = embeds@sw1[800:1150].
    Prefix sums run on the vector engine (tensor_tensor_scan along tokens).
  - Three DRAM tables are written token-major (via PE transposes):
        tabA[t] = A[t]   tabB[t] = B[t]   tabC[t] = [P|Pe][t-1]  (shifted)
    so tabC[start] is the exclusive prefix and tabC[end+1] the inclusive one.
  - Span phase: 4 indirect-DMA row gathers per 128-span tile + small
    elementwise + the [150x150] second layer on PE; the width-bin embedding
    enters via a 5-column multi-hot matmul against a host-folded
    difference table (widths 1..10 only reach bins 1,2,3,4,8).
"""

import numpy as np

# ---- problem constants (hardcoded per contract) ----
T, S = 50000, 100000
DS, DE, H, DW = 400, 350, 150, 20
W_MAX = 10
BINS5 = np.array([1, 2, 3, 4, 8], np.int64)
NCORES = 8
TPC = T // NCORES            # 6250 tokens per core bucket
TL_PAD = 6272                # 49 * 128 padded local tokens (6250 + 9 halo -> 6259)
TBLR = 6400                  # table rows (>= TL_PAD + 1 for the shifted write)
NT_S = 102                   # span tiles per core
SMAX = NT_S * 128            # 13056 padded spans per core (observed max 12737)
TOK_BLOCKS = [(i * 512, 512) for i in range(12)] + [(6144, 128)]
K400 = [(0, 128), (128, 256), (256, 384), (384, 400)]
K350 = [(0, 128), (128, 256), (256, 350)]
K150 = [(0, 128), (128, 150)]

_PROGRAM_CACHE = {}


def _build_program():
    import concourse.bacc as bacc
    import concourse.bass as bass
    import concourse.mybir as mybir
    import concourse.tile as tile
    from concourse.masks import make_identity

    F32 = mybir.dt.float32
    I32 = mybir.dt.int32
    AF = mybir.ActivationFunctionType
    OP = mybir.AluOpType

    nc = bacc.Bacc("TRN2", num_devices=NCORES)

    # ---- I/O ----
    sTd = nc.dram_tensor("sT", [DS, TL_PAD], F32, kind="ExternalInput")
    eTd = nc.dram_tensor("eT", [DE, TL_PAD], F32, kind="ExternalInput")
    aw1m = nc.dram_tensor("aw1m", [DS, 128], F32, kind="ExternalInput")
    sw1am = nc.dram_tensor("sw1am", [DS, 128], F32, kind="ExternalInput")
    sw1bm = nc.dram_tensor("sw1bm", [DS, 128], F32, kind="ExternalInput")
    wl400 = nc.dram_tensor("wl400", [DS, 96], F32, kind="ExternalInput")
    wpmd = nc.dram_tensor("wpm", [DE, 128], F32, kind="ExternalInput")
    wpld = nc.dram_tensor("wpl", [DE, 22], F32, kind="ExternalInput")
    aw2md = nc.dram_tensor("aw2m", [H, 128], F32, kind="ExternalInput")
    aw2ld = nc.dram_tensor("aw2l", [H, 22], F32, kind="ExternalInput")
    aw3d = nc.dram_tensor("aw3", [H, 1], F32, kind="ExternalInput")
    s2md = nc.dram_tensor("s2m", [H, 128], F32, kind="ExternalInput")
    s2ld = nc.dram_tensor("s2l", [H, 22], F32, kind="ExternalInput")
    s3d = nc.dram_tensor("s3", [H, 1], F32, kind="ExternalInput")
    dtabd = nc.dram_tensor("dtab", [5, H], F32, kind="ExternalInput")
    b1d = nc.dram_tensor("bias1", [128, 4], F32, kind="ExternalInput")
    b2cd = nc.dram_tensor("sb2c", [128, 2], F32, kind="ExternalInput")
    b3d = nc.dram_tensor("sb3", [1, 1], F32, kind="ExternalInput")
    a3d = nc.dram_tensor("ab3", [1, 1], F32, kind="ExternalInput")
    startsd = nc.dram_tensor("starts", [128, NT_S], I32, kind="ExternalInput")
    endsd = nc.dram_tensor("ends", [128, NT_S], I32, kind="ExternalInput")
    endsp1d = nc.dram_tensor("endsp1", [128, NT_S], I32, kind="ExternalInput")
    mhd = nc.dram_tensor("mh", [5, SMAX], F32, kind="ExternalInput")
    scoresd = nc.dram_tensor("scores", [1, SMAX], F32, kind="ExternalOutput")

    tabA = nc.dram_tensor("tabA", [TBLR, 150], F32)
    tabB = nc.dram_tensor("tabB", [TBLR, 150], F32)
    tabC = nc.dram_tensor("tabC", [TBLR, 151], F32)

    with tile.TileContext(nc) as tc:
        with (
            tc.tile_pool(name="wpool", bufs=1) as wp,
            tc.tile_pool(name="tok", bufs=2) as tok,
            tc.tile_pool(name="span", bufs=3) as sp,
            tc.tile_pool(name="ps", bufs=6, space="PSUM") as ps,
            tc.tile_pool(name="psc", bufs=2, space="PSUM") as psc,
        ):
            # ---- resident weights / constants ----
            def wload(src, shape, name):
                t = wp.tile(shape, F32, name=name)
                nc.sync.dma_start(t[:], src)
                return t

            w_aw1 = [wload(aw1m[k0:k1, :], [k1 - k0, 128], f"waw1_{i}") for i, (k0, k1) in enumerate(K400)]
            w_sa = [wload(sw1am[k0:k1, :], [k1 - k0, 128], f"wsa_{i}") for i, (k0, k1) in enumerate(K400)]
            w_sb = [wload(sw1bm[k0:k1, :], [k1 - k0, 128], f"wsb_{i}") for i, (k0, k1) in enumerate(K400)]
            w_l4 = [wload(wl400[k0:k1, :], [k1 - k0, 96], f"wl4_{i}") for i, (k0, k1) in enumerate(K400)]
            w_pm = [wload(wpmd[k0:k1, :], [k1 - k0, 128], f"wpm_{i}") for i, (k0, k1) in enumerate(K350)]
            w_pl = [wload(wpld[k0:k1, :], [k1 - k0, 22], f"wpl_{i}") for i, (k0, k1) in enumerate(K350)]
            w_a2m = [wload(aw2md[k0:k1, :], [k1 - k0, 128], f"wa2m_{i}") for i, (k0, k1) in enumerate(K150)]
            w_a2l = [wload(aw2ld[k0:k1, :], [k1 - k0, 22], f"wa2l_{i}") for i, (k0, k1) in enumerate(K150)]
            w_a3 = [wload(aw3d[k0:k1, :], [k1 - k0, 1], f"wa3_{i}") for i, (k0, k1) in enumerate(K150)]
            w_s2m = [wload(s2md[k0:k1, :], [k1 - k0, 128], f"ws2m_{i}") for i, (k0, k1) in enumerate(K150)]
            w_s2l = [wload(s2ld[k0:k1, :], [k1 - k0, 22], f"ws2l_{i}") for i, (k0, k1) in enumerate(K150)]
            w_s3 = [wload(s3d[k0:k1, :], [k1 - k0, 1], f"ws3_{i}") for i, (k0, k1) in enumerate(K150)]
            w_dt = wload(dtabd[:, :], [5, H], "wdt")
            b1 = wload(b1d[:, :], [128, 4], "b1")
            b2c = wload(b2cd[:, :], [128, 2], "b2c")
            b3 = wload(b3d[:, :], [1, 1], "b3")
            a3 = wload(a3d[:, :], [1, 1], "a3")

            starts_sb = wp.tile([128, NT_S], I32, name="starts_sb")
            nc.sync.dma_start(starts_sb[:], startsd[:, :])
            ends_sb = wp.tile([128, NT_S], I32, name="ends_sb")
            nc.sync.dma_start(ends_sb[:], endsd[:, :])
            endsp1_sb = wp.tile([128, NT_S], I32, name="endsp1_sb")
            nc.sync.dma_start(endsp1_sb[:], endsp1d[:, :])

            ones1 = wp.tile([1, 128], F32, name="ones1")
            nc.vector.memset(ones1[:], 1.0)
            ident = wp.tile([128, 128], F32, name="ident")
            make_identity(nc, ident[:])
            zrow = wp.tile([1, 151], F32, name="zrow")
            nc.vector.memset(zrow[:], 0.0)
            nc.sync.dma_start(tabC[0:1, :], zrow[:])

            # ================= token phase =================
            prevC1 = prevC2 = None
            prevTB = 0
            for t0, TB in TOK_BLOCKS:
                st = []
                for i, (k0, k1) in enumerate(K400):
                    s_t = tok.tile([k1 - k0, TB], F32, name=f"s_t{i}", tag=f"st{i}")
                    nc.sync.dma_start(s_t[:], sTd[k0:k1, t0:t0 + TB])
                    st.append(s_t)
                et = []
                for i, (k0, k1) in enumerate(K350):
                    e_t = tok.tile([k1 - k0, TB], F32, name=f"e_t{i}", tag=f"et{i}")
                    nc.sync.dma_start(e_t[:], eTd[k0:k1, t0:t0 + TB])
                    et.append(e_t)

                def mm_group(shape, lhs_list, rhs_list, name):
                    p = ps.tile(shape, F32, name=name, tag="ps")
                    n = len(lhs_list)
                    for i in range(n):
                        nc.tensor.matmul(p[:], lhsT=lhs_list[i][:], rhs=rhs_list[i][:],
                                         start=(i == 0), stop=(i == n - 1))
                    return p

                pH1 = mm_group([128, TB], w_aw1, st, "pH1")
                pA = mm_group([128, TB], w_sa, st, "pA")
                pB = mm_group([128, TB], w_sb, st, "pB")
                pL = mm_group([96, TB], w_l4, st, "pL")
                pEC = mm_group([128, TB], w_pm, et, "pEC")
                pECl = mm_group([22, TB], w_pl, et, "pECl")

                h1a = tok.tile([128, TB], F32, name="h1a", tag="h1a")
                nc.scalar.activation(h1a[:], pH1[:], AF.Relu, bias=b1[0:128, 0:1])
                h1b = tok.tile([22, TB], F32, name="h1b", tag="h1b")
                nc.scalar.activation(h1b[:], pL[0:22, :], AF.Relu, bias=b1[0:22, 1:2])

                pH2 = mm_group([128, TB], w_a2m, [h1a, h1b], "pH2")
                pH2l = mm_group([22, TB], w_a2l, [h1a, h1b], "pH2l")
                h2a = tok.tile([128, TB], F32, name="h2a", tag="h2a")
                nc.scalar.activation(h2a[:], pH2[:], AF.Relu, bias=b1[0:128, 2:3])
                h2b = tok.tile([22, TB], F32, name="h2b", tag="h2b")
                nc.scalar.activation(h2b[:], pH2l[:], AF.Relu, bias=b1[0:22, 3:4])

                pAt = mm_group([1, TB], w_a3, [h2a, h2b], "pAt")
                e_sb = tok.tile([1, TB], F32, name="e_sb", tag="e_sb")
                nc.scalar.activation(e_sb[:], pAt[0:1, :], AF.Exp, bias=a3[0:1, 0:1])

                pBC = ps.tile([128, TB], F32, name="pBC", tag="ps")
                nc.tensor.matmul(pBC[:], lhsT=ones1[:], rhs=e_sb[:], start=True, stop=True)
                ebc = tok.tile([128, TB], F32, name="ebc", tag="ebc")
                nc.any.tensor_copy(ebc[:], pBC[:])

                EV1 = tok.tile([128, TB], F32, name="EV1", tag="EV1")
                nc.vector.tensor_mul(EV1[:], pEC[:], ebc[:])
                EV2 = tok.tile([33, TB], F32, name="EV2", tag="EV2")
                nc.vector.memset(EV2[:], 0.0)
                nc.vector.tensor_mul(EV2[0:22, :], pECl[:], ebc[0:22, :])
                nc.any.tensor_copy(EV2[32:33, :], e_sb[:])

                C1 = tok.tile([128, TB], F32, name="C1", tag="c1")
                init1 = 0.0 if prevC1 is None else prevC1[:, prevTB - 1:prevTB]
                nc.vector.tensor_tensor_scan(C1[:], EV1[:], EV1[:], init1,
                                             op0=OP.add, op1=OP.bypass)
                C2 = tok.tile([33, TB], F32, name="C2", tag="c2")
                init2 = 0.0 if prevC2 is None else prevC2[:, prevTB - 1:prevTB]
                nc.vector.tensor_tensor_scan(C2[:], EV2[:], EV2[:], init2,
                                             op0=OP.add, op1=OP.bypass)
                prevC1, prevC2, prevTB = C1, C2, TB

                packed = tok.tile([128, TB], F32, name="packed", tag="packed")
                nc.any.tensor_copy(packed[0:22, :], pL[32:54, :])
                nc.any.tensor_copy(packed[32:54, :], pL[64:86, :])
                nc.any.tensor_copy(packed[64:86, :], C2[0:22, :])
                nc.any.tensor_copy(packed[96:97, :], C2[32:33, :])

                A1 = tok.tile([128, TB], F32, name="A1", tag="A1")
                nc.any.tensor_copy(A1[:], pA[:])
                B1 = tok.tile([128, TB], F32, name="B1", tag="B1")
                nc.any.tensor_copy(B1[:], pB[:])

                for j in range(TB // 128):
                    js = j * 128
                    r0 = t0 + js

                    def tr(src_ap, name):
                        pt = ps.tile([128, 128], F32, name=f"pt_{name}", tag="ps")
                        nc.tensor.transpose(pt[:], src_ap, ident[:])
                        return pt

                    pta = tr(A1[:, js:js + 128], "a1")
                    ptb = tr(B1[:, js:js + 128], "b1")
                    ptc = tr(C1[:, js:js + 128], "c1")
                    ptp = tr(packed[:, js:js + 128], "pk")

                    afull = tok.tile([128, 150], F32, name="afull", tag="afull")
                    nc.any.tensor_copy(afull[:, 0:128], pta[:])
                    nc.any.tensor_copy(afull[:, 128:150], ptp[:, 0:22])
                    bfull = tok.tile([128, 150], F32, name="bfull", tag="bfull")
                    nc.any.tensor_copy(bfull[:, 0:128], ptb[:])
                    nc.any.tensor_copy(bfull[:, 128:150], ptp[:, 32:54])
                    cfull = tok.tile([128, 151], F32, name="cfull", tag="cfull")
                    nc.any.tensor_copy(cfull[:, 0:128], ptc[:])
                    nc.any.tensor_copy(cfull[:, 128:150], ptp[:, 64:86])
                    nc.any.tensor_copy(cfull[:, 150:151], ptp[:, 96:97])

                    nc.sync.dma_start(tabA[r0:r0 + 128, :], afull[:])
                    nc.sync.dma_start(tabB[r0:r0 + 128, :], bfull[:])
                    nc.sync.dma_start(tabC[r0 + 1:r0 + 129, :], cfull[:])

            # ================= span phase =================
            pSC = None
            for k in range(NT_S):
                g, j = divmod(k, 4)

                def gather(tab, idx_sb, width, name):
                    gt = sp.tile([128, width], F32, name=name, tag=name)
                    nc.gpsimd.indirect_dma_start(
                        out=gt[:], out_offset=None, in_=tab[:, :],
                        in_offset=bass.IndirectOffsetOnAxis(ap=idx_sb[:, k:k + 1], axis=0))
                    return gt

                GA = gather(tabA, starts_sb, 150, "GA")
                GC0 = gather(tabC, starts_sb, 151, "GC0")
                GB = gather(tabB, ends_sb, 150, "GB")
                GC1 = gather(tabC, endsp1_sb, 151, "GC1")

                mhs = sp.tile([5, 128], F32, name="mhs", tag="mhs")
                nc.sync.dma_start(mhs[:], mhd[:, k * 128:(k + 1) * 128])
                psw = ps.tile([128, H], F32, name="psw", tag="ps")
                nc.tensor.matmul(psw[:], lhsT=mhs[:], rhs=w_dt[:], start=True, stop=True)

                diff = sp.tile([128, 151], F32, name="diff", tag="diff")
                nc.vector.tensor_sub(diff[:], GC1[:], GC0[:])
                rec = sp.tile([128, 1], F32, name="rec", tag="rec")
                nc.vector.reciprocal(rec[:], diff[:, 150:151])
                t1 = sp.tile([128, H], F32, name="t1", tag="t1")
                nc.vector.scalar_tensor_tensor(t1[:], diff[:, 0:150], rec[:, 0:1],
                                               GA[:], op0=OP.mult, op1=OP.add)
                t2 = sp.tile([128, H], F32, name="t2", tag="t2")
                nc.vector.tensor_add(t2[:], t1[:], GB[:])
                h1p = sp.tile([128, H], F32, name="h1p", tag="h1p")
                nc.vector.tensor_add(h1p[:], t2[:], psw[:])
                h1s = sp.tile([128, H], F32, name="h1s", tag="h1s")
                nc.scalar.activation(h1s[:], h1p[:], AF.Relu)

                pTa = ps.tile([128, 128], F32, name="pTa", tag="ps")
                nc.tensor.transpose(pTa[:], h1s[:, 0:128], ident[:])
                pTb = ps.tile([22, 128], F32, name="pTb", tag="ps")
                nc.tensor.transpose(pTb[:], h1s[:, 128:150], ident[:])
                h1t1 = sp.tile([128, 128], F32, name="h1t1", tag="h1t1")
                nc.any.tensor_copy(h1t1[:], pTa[:])
                h1t2 = sp.tile([22, 128], F32, name="h1t2", tag="h1t2")
                nc.any.tensor_copy(h1t2[:], pTb[:])

                pH2a = ps.tile([128, 128], F32, name="pH2a", tag="ps")
                nc.tensor.matmul(pH2a[:], lhsT=w_s2m[0][:], rhs=h1t1[:], start=True, stop=False)
                nc.tensor.matmul(pH2a[:], lhsT=w_s2m[1][:], rhs=h1t2[:], start=False, stop=True)
                pH2b = ps.tile([22, 128], F32, name="pH2b", tag="ps")
                nc.tensor.matmul(pH2b[:], lhsT=w_s2l[0][:], rhs=h1t1[:], start=True, stop=False)
                nc.tensor.matmul(pH2b[:], lhsT=w_s2l[1][:], rhs=h1t2[:], start=False, stop=True)

                h2t1 = sp.tile([128, 128], F32, name="h2t1", tag="h2t1")
                nc.scalar.activation(h2t1[:], pH2a[:], AF.Relu, bias=b2c[0:128, 0:1])
                h2t2 = sp.tile([22, 128], F32, name="h2t2", tag="h2t2")
                nc.scalar.activation(h2t2[:], pH2b[:], AF.Relu, bias=b2c[0:22, 1:2])

                if j == 0:
                    pSC = psc.tile([1, 512], F32, name="pSC", tag="sc")
                sl = pSC[0:1, j * 128:(j + 1) * 128]
                nc.tensor.matmul(sl, lhsT=w_s3[0][:], rhs=h2t1[:], start=True, stop=False)
                nc.tensor.matmul(sl, lhsT=w_s3[1][:], rhs=h2t2[:], start=False, stop=True)

                if j == 3 or k == NT_S - 1:
                    width = (j + 1) * 128
                    scs = sp.tile([1, 512], F32, name="scs", tag="scs")
                    nc.scalar.activation(scs[0:1, 0:width], pSC[0:1, 0:width],
                                         AF.Identity, bias=b3[0:1, 0:1])
                    nc.sync.dma_start(scoresd[0:1, g * 512:g * 512 + width],
                                      scs[0:1, 0:width])
    nc.compile()
    return nc


def _prep_shared(inputs):
    """Host-side weight packing (pure layout prep, shared by all cores)."""
    f32 = lambda x: np.ascontiguousarray(np.asarray(x), dtype=np.float32)
    aw1, ab1 = f32(inputs["aw1"]), f32(inputs["ab1"])
    aw2, ab2 = f32(inputs["aw2"]), f32(inputs["ab2"])
    aw3, ab3 = f32(inputs["aw3"]), f32(inputs["ab3"])
    sw1, sb1 = f32(inputs["sw1"]), f32(inputs["sb1"])
    sw2, sb2 = f32(inputs["sw2"]), f32(inputs["sb2"])
    sw3, sb3 = f32(inputs["sw3"]), f32(inputs["sb3"])
    wt = f32(inputs["width_table"])

    sw1a, sw1b, sw1p, sw1w = sw1[0:400], sw1[400:800], sw1[800:1150], sw1[1150:1170]

    wl = np.zeros((DS, 96), np.float32)
    wl[:, 0:22] = aw1[:, 128:150]
    wl[:, 32:54] = sw1a[:, 128:150]
    wl[:, 64:86] = sw1b[:, 128:150]

    # width-bin difference table with sb1 folded in (widths are 1..10 -> bin 1..5)
    Wmb = wt @ sw1w  # [9, 150]
    dtab = np.zeros((5, H), np.float32)
    dtab[0] = Wmb[1] + sb1
    for jj in range(1, 5):
        dtab[jj] = Wmb[jj + 1] - Wmb[jj]

    b1p = np.zeros((128, 4), np.float32)
    b1p[:, 0] = ab1[0:128]
    b1p[0:22, 1] = ab1[128:150]
    b1p[:, 2] = ab2[0:128]
    b1p[0:22, 3] = ab2[128:150]
    b2cp = np.zeros((128, 2), np.float32)
    b2cp[:, 0] = sb2[0:128]
    b2cp[0:22, 1] = sb2[128:150]

    c = np.ascontiguousarray
    return {
        "aw1m": c(aw1[:, 0:128]), "sw1am": c(sw1a[:, 0:128]), "sw1bm": c(sw1b[:, 0:128]),
        "wl400": wl, "wpm": c(sw1p[:, 0:128]), "wpl": c(sw1p[:, 128:150]),
        "aw2m": c(aw2[:, 0:128]), "aw2l": c(aw2[:, 128:150]), "aw3": c(aw3),
        "s2m": c(sw2[:, 0:128]), "s2l": c(sw2[:, 128:150]), "s3": c(sw3),
        "dtab": dtab, "bias1": b1p, "sb2c": b2cp,
        "sb3": sb3.reshape(1, 1).astype(np.float32),
        "ab3": ab3.reshape(1, 1).astype(np.float32),
    }


def prepare_in_maps(inputs):
    """Host-side sharding: returns (in_maps, sels) — per-core input dicts and
    the original span indices each core's padded slots map back to."""
    states = np.asarray(inputs["states"], dtype=np.float32)
    embeds = np.asarray(inputs["embeds"], dtype=np.float32)
    starts = np.asarray(inputs["span_starts"]).astype(np.int64)
    widths = np.asarray(inputs["span_widths"]).astype(np.int64)

    shared = _prep_shared(inputs)

    bucket = np.minimum(starts // TPC, NCORES - 1)
    order = np.argsort(bucket, kind="stable")
    counts = np.bincount(bucket, minlength=NCORES)
    assert counts.max() <= SMAX, f"span bucket overflow: {counts.max()} > {SMAX}"
    offs = np.zeros(NCORES + 1, np.int64)
    offs[1:] = np.cumsum(counts)

    mh_full = (widths[None, :] >= BINS5[:, None]).astype(np.float32)  # [5, S]

    in_maps = []
    sels = []
    for cix in range(NCORES):
        t0 = cix * TPC
        tl = min(T, t0 + TPC + W_MAX - 1) - t0
        sT = np.zeros((DS, TL_PAD), np.float32)
        sT[:, :tl] = states[t0:t0 + tl].T
        eT = np.zeros((DE, TL_PAD), np.float32)
        eT[:, :tl] = embeds[t0:t0 + tl].T

        sel = order[offs[cix]:offs[cix + 1]]
        sels.append(sel)
        n = len(sel)
        ls = np.zeros(SMAX, np.int32)
        le = np.zeros(SMAX, np.int32)
        ls[:n] = (starts[sel] - t0).astype(np.int32)
        le[:n] = ls[:n] + widths[sel].astype(np.int32) - 1
        mh = np.zeros((5, SMAX), np.float32)
        mh[:, :n] = mh_full[:, sel]

        pm = lambda a: np.ascontiguousarray(a.reshape(NT_S, 128).T)
        in_maps.append({
            "sT": sT, "eT": eT,
            "starts": pm(ls), "ends": pm(le), "endsp1": pm(le + 1),
            "mh": mh,
            **shared,
        })
    return in_maps, sels


def kernel(**inputs) -> np.ndarray:
    in_maps, sels = prepare_in_maps(inputs)

    if "nc" not in _PROGRAM_CACHE:
        _PROGRAM_CACHE["nc"] = _build_program()
    nc = _PROGRAM_CACHE["nc"]

    from concourse.bass_utils import run_bass_kernel_spmd
    res = run_bass_kernel_spmd(nc, in_maps, core_ids=list(range(NCORES)))
    _PROGRAM_CACHE["last_res"] = res  # exec_time_ns etc, for the test harness

    out = np.zeros(S, np.float32)
    for cix in range(NCORES):
        n = len(sels[cix])
        vals = np.asarray(res.results[cix]["scores"]).reshape(-1)[:n]
        out[sels[cix]] = vals
    return out
